# revision 18
# baseline (speedup 1.0000x reference)
"""Trainium2 Bass kernel for nn_CSIW (prototype-scaling network).

Data-parallel over qry batch (4096 -> 8 cores x 512), spt + params
replicated. Per core: 4 batch-tiles of 128 rows (partition = batch,
free = (c, m) = 640*25). Engine split per tile:
  DVE    : mean-over-m reduce, qs = q*qg (bcast TT, in place), dist reduce-m
  ACT    : per-m fused Square(qs - q2) via per-partition bias AP, per-m
           sum-over-c via accum_out, tanh/sigmoid, PSUM->SBUF copies
  GPSIMD : max-over-c via strided tensor_tensor max tree
  PE     : all matmuls (BN-folded linears; conv as [50,25] matmul against a
           host-built kernel matrix; transposes; output 0.5wp+0.5wq via
           identity-copy + K=1 ones matmuls into PSUM, DMA'd straight out)
The tiny prt (support) branch runs once per core in channel-major layout.
"""

import numpy as np
from contextlib import ExitStack

C = 640
M = 25
WS = 25          # way * shot
B = 4096
NCORES = 8
BS = B // NCORES # 512 batch rows per core
NB = 128         # batch rows per tile
NT = BS // NB    # 4 tiles
CH = 320         # cout half for matmul free-dim chunks
CC = 5           # 640 / 128 channel chunks
EPS = 1e-5

_PROG_CACHE = {}


def _build_program(repeat=1):
    import concourse.bass as bass
    import concourse.bacc as bacc
    import concourse.tile as tile
    from concourse import mybir

    f32 = mybir.dt.float32
    Alu = mybir.AluOpType
    Act = mybir.ActivationFunctionType
    AX = mybir.AxisListType

    nc = bacc.Bacc("TRN2", target_bir_lowering=False, debug=False)

    # ---- DRAM I/O ----
    q_d = nc.dram_tensor("q", [BS, C * M], f32, kind="ExternalInput").ap()
    sptT_d = nc.dram_tensor("sptT", [C, WS * M], f32, kind="ExternalInput").ap()
    wq_d = nc.dram_tensor("wq", [C, C], f32, kind="ExternalInput").ap()
    ws_d = nc.dram_tensor("ws", [C, C], f32, kind="ExternalInput").ap()
    wp_d = nc.dram_tensor("wp", [C, C], f32, kind="ExternalInput").ap()
    wpr_d = nc.dram_tensor("wpr", [C, C], f32, kind="ExternalInput").ap()
    bias_d = nc.dram_tensor("biases", [4, C], f32, kind="ExternalInput").ap()
    kq_d = nc.dram_tensor("kq", [64, M], f32, kind="ExternalInput").ap()
    kp_d = nc.dram_tensor("kp", [64, M], f32, kind="ExternalInput").ap()
    ones_d = nc.dram_tensor("ones_row", [1, 128], f32, kind="ExternalInput").ap()
    onesc_d = nc.dram_tensor("ones_col", [128, 1], f32, kind="ExternalInput").ap()
    ident_d = nc.dram_tensor("ident", [128, 128], f32, kind="ExternalInput").ap()
    cb_d = nc.dram_tensor("cb", [128, 2], f32, kind="ExternalInput").ap()
    out_d = nc.dram_tensor("out", [WS, BS, C], f32, kind="ExternalOutput").ap()

    with tile.TileContext(nc) as tc, ExitStack() as ctx:
        cpool = ctx.enter_context(tc.tile_pool(name="consts", bufs=1))
        qpool = ctx.enter_context(tc.tile_pool(name="qdata", bufs=2))

        def cld(shape, name, src):
            t = cpool.tile(shape, f32, name=name, tag=name)
            nc.sync.dma_start(t[:], src)
            return t

        ones_sb = cld([1, 128], "ones_sb", ones_d[:, :])
        onesc_sb = cld([128, 1], "onesc_sb", onesc_d[:, :])
        ident_sb = cld([128, 128], "ident_sb", ident_d[:, :])
        kq_sb = cld([64, 25], "kq_sb", kq_d[:, :])
        kp_sb = cld([64, 25], "kp_sb", kp_d[:, :])
        cb_sb = cld([128, 2], "cb_sb", cb_d[:, :])
        bias_t = []
        for i in range(4):
            bt = cpool.tile([1, C], f32, name=f"bias{i}", tag=f"bias{i}")
            nc.sync.dma_start(bt[0:1, :], bias_d[i:i + 1, :])
            bias_t.append(bt)

        wq_sb = cpool.tile([128, CC * C], f32, name="wq_sb", tag="wq_sb")
        nc.sync.dma_start(
            wq_sb[:].rearrange("p (a n) -> p a n", a=CC),
            wq_d.rearrange("(a p) n -> p a n", p=128),
        )
        ws_sb = cpool.tile([128, CC * C], f32, name="ws_sb", tag="ws_sb")
        nc.sync.dma_start(
            ws_sb[:].rearrange("p (a n) -> p a n", a=CC),
            ws_d.rearrange("(a p) n -> p a n", p=128),
        )
        wph = cpool.tile([WS, C], f32, name="wph", tag="wph")  # 0.5*w_prt

        # =========================================================
        # prt (support) branch, channel-major chunks [128c, (s,m)]
        # =========================================================
        with tc.tile_pool(name="prts", bufs=1) as ppool, \
             tc.tile_pool(name="pp_t", bufs=2, space="PSUM") as pp_t, \
             tc.tile_pool(name="pp_mm", bufs=2, space="PSUM") as pp_mm, \
             tc.tile_pool(name="pp_out", bufs=4, space="PSUM") as pp_out:

            wfull = ppool.tile([128, CC * C], f32, name="wfull", tag="wfull")
            nc.sync.dma_start(
                wfull[:].rearrange("p (a n) -> p a n", a=CC),
                wp_d.rearrange("(a p) n -> p a n", p=128),
            )
            wf3 = wfull[:].rearrange("p (a n) -> p a n", a=CC)
            P_sb = ppool.tile([128, CC * WS * M], f32, name="P_sb", tag="P_sb")
            nc.sync.dma_start(
                P_sb[:].rearrange("p (a f) -> p a f", a=CC),
                sptT_d.rearrange("(a p) f -> p a f", p=128),
            )
            P3 = P_sb[:].rearrange("p (a f) -> p a f", a=CC)        # [128,5,625]
            P4 = P_sb[:].rearrange("p (a s m) -> p a s m", a=CC, s=WS)

            # mean over m (1/25 folded into wp host-side)
            xqp = ppool.tile([128, CC * WS], f32, name="xqp", tag="xqp")
            for cc in range(CC):
                nc.vector.reduce_sum(
                    xqp[:, cc * WS:(cc + 1) * WS], P4[:, cc], axis=AX.X)

            # qg_p = 1 + tanh(Wp @ xq + b), channel-major [128c, 25s]
            qgp = ppool.tile([128, CC * WS], f32, name="qgp", tag="qgp")
            for co in range(CC):
                ps_g = pp_mm.tile([128, WS], f32, name="ps_g", tag="mm")
                for ci in range(CC):
                    nc.tensor.matmul(
                        ps_g[:], wf3[:, ci, co * 128:(co + 1) * 128],
                        xqp[:, ci * WS:(ci + 1) * WS],
                        start=(ci == 0), stop=False)
                nc.tensor.matmul(
                    ps_g[:], bias_t[2][0:1, co * 128:(co + 1) * 128],
                    ones_sb[0:1, 0:WS], start=False, stop=True)
                tgp = ppool.tile([128, WS], f32, name="tgp", tag="tgp")
                nc.scalar.activation(tgp[:], ps_g[:], Act.Tanh)
                nc.vector.tensor_scalar(
                    qgp[:, co * WS:(co + 1) * WS], tgp[:], 1.0, None, Alu.add)

            # qs_p = prt * qg (in place), bcast over m
            for cc in range(CC):
                nc.vector.tensor_tensor(
                    P4[:, cc], P4[:, cc],
                    qgp[:, cc * WS:(cc + 1) * WS].unsqueeze(2).broadcast_to(
                        [128, WS, M]),
                    op=Alu.mult)

            # max over c: cross-chunk folds + partition tree (ping-pong)
            ms = ppool.tile([128, WS * M], f32, name="ms", tag="ms")
            mt = ppool.tile([128, WS * M], f32, name="mt", tag="mt")
            nc.vector.tensor_tensor(ms[:], P3[:, 0], P3[:, 1], op=Alu.max)
            nc.vector.tensor_tensor(mt[:], ms[:], P3[:, 2], op=Alu.max)
            nc.vector.tensor_tensor(ms[:], mt[:], P3[:, 3], op=Alu.max)
            nc.vector.tensor_tensor(mt[:], ms[:], P3[:, 4], op=Alu.max)
            # partition reduce via gpsimd (walrus forbids partition-offset
            # TensorTensor operands)
            from concourse import bass_isa
            nc.gpsimd.partition_all_reduce(
                ms[:], mt[:], channels=128, reduce_op=bass_isa.ReduceOp.max)
            cur = ms
            # cur[0:1, :] = channel max, layout (s, m)

            # sum over c via PE ones-column matmuls -> [1, 625]
            ps_s1 = pp_mm.tile([1, 320], f32, name="ps_s1", tag="mm")
            for cc in range(CC):
                nc.tensor.matmul(ps_s1[:], onesc_sb[:, 0:1], P3[:, cc, 0:320],
                                 start=(cc == 0), stop=(cc == CC - 1))
            ps_s2 = pp_mm.tile([1, WS * M - 320], f32, name="ps_s2", tag="mm")
            for cc in range(CC):
                nc.tensor.matmul(ps_s2[:], onesc_sb[:, 0:1], P3[:, cc, 320:],
                                 start=(cc == 0), stop=(cc == CC - 1))
            sumrow = ppool.tile([1, WS * M], f32, name="sumrow", tag="sumrow")
            nc.scalar.copy(sumrow[0:1, 0:320], ps_s1[:])
            nc.scalar.copy(sumrow[0:1, 320:], ps_s2[:])

            # pooledT [50, 25]: rows 0-24 max over (q=m), rows 25-49 sum
            # route: flat [1,625] -> [25s,25q] DMA -> PE transpose -> [25q,25s]
            pooledT_p = ppool.tile([64, WS], f32, name="pooledT_p",
                                   tag="pooledT_p")
            tmp_mx = ppool.tile([WS, M], f32, name="tmp_mx", tag="tmp_mx")
            nc.sync.dma_start(tmp_mx[:, :], cur[0:1, :])
            tmp_sm = ppool.tile([WS, M], f32, name="tmp_sm", tag="tmp_sm")
            nc.sync.dma_start(tmp_sm[:, :], sumrow[0:1, :])
            nc.gpsimd.memset(pooledT_p[:, :], 0.0)
            ps_tm = pp_t.tile([M, WS], f32, name="ps_tm", tag="t")
            nc.tensor.transpose(ps_tm[:], tmp_mx[:, :], ident_sb[0:WS, 0:M])
            nc.scalar.copy(pooledT_p[0:WS, :], ps_tm[:])
            ps_ts = pp_t.tile([M, WS], f32, name="ps_ts", tag="t")
            nc.tensor.transpose(ps_ts[:], tmp_sm[:, :], ident_sb[0:WS, 0:M])
            nc.scalar.copy(pooledT_p[32:32 + WS, :], ps_ts[:])

            # conv as matmul -> [25s, 25p]; sigmoid(+cb)
            ps_c = pp_mm.tile([WS, M], f32, name="ps_c", tag="mm")
            nc.tensor.matmul(ps_c[:], pooledT_p[:, :], kp_sb[:, :],
                             start=True, stop=True)
            prt2sb = ppool.tile([WS, M], f32, name="prt2sb", tag="prt2sb")
            nc.scalar.activation(prt2sb[:], ps_c[:], Act.Sigmoid,
                                 bias=cb_sb[0:WS, 1:2])

            # flatten [25,25] -> [1,625]; broadcast to 128 partitions via PE
            prow = ppool.tile([1, WS * M], f32, name="prow", tag="prow")
            nc.sync.dma_start(prow[0:1, :], prt2sb[:, :])
            ps_b1 = pp_out.tile([128, 320], f32, name="ps_b1", tag="o")
            nc.tensor.matmul(ps_b1[:], ones_sb[0:1, :], prow[0:1, 0:320],
                             start=True, stop=True)
            ps_b2 = pp_out.tile([128, WS * M - 320], f32, name="ps_b2", tag="o")
            nc.tensor.matmul(ps_b2[:], ones_sb[0:1, :], prow[0:1, 320:],
                             start=True, stop=True)
            prt2b = ppool.tile([128, WS * M], f32, name="prt2b", tag="prt2b")
            nc.scalar.copy(prt2b[:, 0:320], ps_b1[:])
            nc.scalar.copy(prt2b[:, 320:], ps_b2[:])

            # dist1[c, s] = sum_m (qs_p - prt2b)^2
            dist1 = ppool.tile([128, CC * WS], f32, name="dist1", tag="dist1")
            for cc in range(CC):
                nc.vector.tensor_tensor(ms[:], P3[:, cc], prt2b[:],
                                        op=Alu.subtract)
                nc.scalar.activation(mt[:], ms[:], Act.Square)
                nc.vector.reduce_sum(
                    dist1[:, cc * WS:(cc + 1) * WS],
                    mt[:].rearrange("p (s m) -> p s m", s=WS), axis=AX.X)

            # w_prt half = 0.5 + 0.5*tanh(Wpr_neg @ dist1 + b)
            nc.sync.dma_start(
                wfull[:].rearrange("p (a n) -> p a n", a=CC),
                wpr_d.rearrange("(a p) n -> p a n", p=128),
            )
            wphT = ppool.tile([128, CC * WS], f32, name="wphT", tag="wphT")
            for co in range(CC):
                ps_w = pp_mm.tile([128, WS], f32, name="ps_w", tag="mm")
                for ci in range(CC):
                    nc.tensor.matmul(
                        ps_w[:], wf3[:, ci, co * 128:(co + 1) * 128],
                        dist1[:, ci * WS:(ci + 1) * WS],
                        start=(ci == 0), stop=False)
                nc.tensor.matmul(
                    ps_w[:], bias_t[3][0:1, co * 128:(co + 1) * 128],
                    ones_sb[0:1, 0:WS], start=False, stop=True)
                tpr = ppool.tile([128, WS], f32, name="tpr", tag="tpr")
                nc.scalar.activation(tpr[:], ps_w[:], Act.Tanh)
                nc.vector.tensor_scalar(
                    wphT[:, co * WS:(co + 1) * WS], tpr[:], 0.5, 0.5,
                    Alu.mult, Alu.add)
            # transpose [128c, 25s] chunks -> wph [25, 640]
            for cc in range(CC):
                ps_tp = pp_t.tile([WS, 128], f32, name="ps_tp", tag="t")
                nc.tensor.transpose(ps_tp[:], wphT[:, cc * WS:(cc + 1) * WS],
                                    ident_sb[:])
                nc.scalar.copy(wph[0:WS, cc * 128:(cc + 1) * 128], ps_tp[:])

        # =========================================================
        # batch tiles
        # =========================================================
        with tc.tile_pool(name="main", bufs=2) as mpool, \
             tc.tile_pool(name="bp_t", bufs=2, space="PSUM") as bp_t, \
             tc.tile_pool(name="bp_mm", bufs=2, space="PSUM") as bp_mm, \
             tc.tile_pool(name="bp_out", bufs=4, space="PSUM") as bp_out, \
             ExitStack() as rep_ctx:

            wq3 = wq_sb[:].rearrange("p (a n) -> p a n", a=CC)
            ws3 = ws_sb[:].rearrange("p (a n) -> p a n", a=CC)

            for ti in [t for _ in range(repeat) for t in range(NT)]:
                Q = qpool.tile([128, C * M], f32, name="Q", tag="Q")
                for ck in range(4):
                    nc.sync.dma_start(
                        Q[:, ck * 4000:(ck + 1) * 4000],
                        q_d[ti * NB:(ti + 1) * NB, ck * 4000:(ck + 1) * 4000])
                Q3 = Q[:].rearrange("p (c m) -> p c m", m=M)   # [128,640,25]

                # xq = sum_m q  (1/25 folded into wq host-side)
                xq = mpool.tile([128, C], f32, name="xq", tag="xq")
                nc.vector.reduce_sum(xq[:], Q3, axis=AX.X)
                # transpose xq -> xqT chunks [128c, 128b]
                xqT = mpool.tile([128, C], f32, name="xqT", tag="xqT", bufs=1)
                for cc in range(CC):
                    ps_t = bp_t.tile([128, 128], f32, name="ps_t", tag="t")
                    nc.tensor.transpose(
                        ps_t[:], xq[:, cc * 128:(cc + 1) * 128], ident_sb[:])
                    nc.scalar.copy(xqT[:, cc * 128:(cc + 1) * 128], ps_t[:])
                # qg = 1 + tanh(xq @ wq + b)
                qg = mpool.tile([128, C], f32, name="qg", tag="qg")
                for h in range(2):
                    ps_g = bp_mm.tile([128, CH], f32, name="ps_g", tag="mm")
                    for cc in range(CC):
                        nc.tensor.matmul(
                            ps_g[:], xqT[:, cc * 128:(cc + 1) * 128],
                            wq3[:, cc, h * CH:(h + 1) * CH],
                            start=(cc == 0), stop=False)
                    nc.tensor.matmul(
                        ps_g[:], ones_sb[0:1, 0:128],
                        bias_t[0][0:1, h * CH:(h + 1) * CH],
                        start=False, stop=True)
                    nc.scalar.activation(qg[:, h * CH:(h + 1) * CH], ps_g[:],
                                         Act.Tanh)
                nc.vector.tensor_scalar(qg[:], qg[:], 1.0, None, Alu.add)

                # qs = q * qg (in place, bcast over m) on gpsimd
                nc.gpsimd.tensor_tensor(
                    Q3, Q3, qg[:].unsqueeze(2).broadcast_to([128, C, M]),
                    op=Alu.mult)

                # pooled [128, 50]: cols 0-24 max-over-c, cols 25-49 sum
                pooled = mpool.tile([128, 64], f32, name="pooled", tag="pooled")
                nc.gpsimd.memset(pooled[:, :], 0.0)
                # max over c: strided reduce, view (m outer, c inner)
                Qmc = Q[:].rearrange("p (c m) -> p m c", m=M)
                nc.vector.reduce_max(pooled[:, 0:M], Qmc, axis=AX.X)
                # sum over c: 25 ACT copies with accum_out
                ascr = mpool.tile([128, C], f32, name="ascr", tag="ascr",
                                  bufs=1)
                for m in range(M):
                    nc.scalar.activation(
                        ascr[:], Q3[:, :, m], Act.Copy,
                        accum_out=pooled[:, 32 + m:32 + m + 1])

                # pooledT = pooled.T -> conv matmul -> q2neg
                ps_pt = bp_t.tile([64, 128], f32, name="ps_pt", tag="t")
                nc.tensor.transpose(ps_pt[:], pooled[:], ident_sb[:])
                pooledT = mpool.tile([64, 128], f32, name="pooledT",
                                     tag="pooledT")
                nc.scalar.copy(pooledT[:], ps_pt[:])
                ps_cv = bp_mm.tile([128, M], f32, name="ps_cv", tag="mm")
                nc.tensor.matmul(ps_cv[:], pooledT[:], kq_sb[:, :],
                                 start=True, stop=True)
                q2t = mpool.tile([128, M], f32, name="q2t", tag="q2t")
                nc.scalar.activation(q2t[:], ps_cv[:], Act.Sigmoid,
                                     bias=cb_sb[:, 0:1])
                q2neg = mpool.tile([128, M], f32, name="q2neg", tag="q2neg")
                nc.vector.tensor_scalar(q2neg[:], q2t[:], -1.0, None, Alu.mult)

                # dist: per-m fused (qs - q2)^2 in place on Q
                for m in range(M):
                    nc.scalar.activation(
                        Q3[:, :, m], Q3[:, :, m], Act.Square,
                        bias=q2neg[:, m:m + 1])
                dist2 = mpool.tile([128, C], f32, name="dist2", tag="dist2")
                nc.vector.reduce_sum(dist2[:], Q3, axis=AX.X)

                # transpose dist2; wqh = 0.5 + 0.5*tanh(ws_neg @ dist2 + b)
                dist2T = mpool.tile([128, C], f32, name="dist2T", tag="dist2T",
                                    bufs=1)
                for cc in range(CC):
                    ps_t2 = bp_t.tile([128, 128], f32, name="ps_t2", tag="t")
                    nc.tensor.transpose(
                        ps_t2[:], dist2[:, cc * 128:(cc + 1) * 128],
                        ident_sb[:])
                    nc.scalar.copy(dist2T[:, cc * 128:(cc + 1) * 128],
                                   ps_t2[:])
                wqh = mpool.tile([128, C], f32, name="wqh", tag="wqh")
                for h in range(2):
                    ps_w = bp_mm.tile([128, CH], f32, name="ps_w", tag="mm")
                    for cc in range(CC):
                        nc.tensor.matmul(
                            ps_w[:], dist2T[:, cc * 128:(cc + 1) * 128],
                            ws3[:, cc, h * CH:(h + 1) * CH],
                            start=(cc == 0), stop=False)
                    nc.tensor.matmul(
                        ps_w[:], ones_sb[0:1, 0:128],
                        bias_t[1][0:1, h * CH:(h + 1) * CH],
                        start=False, stop=True)
                    nc.scalar.activation(wqh[:, h * CH:(h + 1) * CH], ps_w[:],
                                         Act.Tanh)
                nc.vector.tensor_scalar(wqh[:], wqh[:], 0.5, 0.5,
                                        Alu.mult, Alu.add)

                # out[s, b, c] = wqh[b, c] + wph[s, c]; PE -> PSUM -> DRAM
                for s in range(WS):
                    wrow = mpool.tile([1, C], f32, name="wrow", tag="wrow",
                                      bufs=2)
                    nc.sync.dma_start(wrow[0:1, :], wph[s:s + 1, :])
                    out_sb = mpool.tile([128, C], f32, name="out_sb",
                                        tag="out_sb", bufs=3)
                    for h in range(2):
                        ps_o = bp_out.tile([128, CH], f32, name="ps_o",
                                           tag="o")
                        nc.tensor.matmul(ps_o[:], ident_sb[:],
                                         wqh[:, h * CH:(h + 1) * CH],
                                         start=True, stop=False)
                        nc.tensor.matmul(ps_o[:], ones_sb[0:1, :],
                                         wrow[0:1, h * CH:(h + 1) * CH],
                                         start=False, stop=True)
                        if s % 2 == 0:
                            nc.vector.tensor_copy(
                                out_sb[:, h * CH:(h + 1) * CH], ps_o[:])
                        else:
                            nc.scalar.copy(
                                out_sb[:, h * CH:(h + 1) * CH], ps_o[:])
                    nc.sync.dma_start(
                        out_d[s, ti * NB:(ti + 1) * NB, :], out_sb[:])
    nc.compile()
    return nc


def _prep_host(inputs):
    """Fold BN into weights, build conv kernel matrices, layout transforms."""
    def fold(W, g, b, rm, rv, scale=1.0, neg=False):
        W = np.asarray(W, np.float32)
        g = np.asarray(g, np.float32)
        b = np.asarray(b, np.float32)
        rm = np.asarray(rm, np.float32)
        rv = np.asarray(rv, np.float32)
        alpha = g / np.sqrt(rv + EPS)
        Wt = (W.T * alpha[None, :]) * scale       # [cin, cout]
        if neg:
            Wt = -Wt
        bt = b - rm * alpha
        return np.ascontiguousarray(Wt, np.float32), bt.astype(np.float32)

    wq, bq = fold(inputs["W_q"], inputs["g_q"], inputs["b_q"],
                  inputs["rm_q"], inputs["rv_q"], scale=1.0 / M)
    ws, bs = fold(inputs["W_qry_s"], inputs["g_qry_s"], inputs["b_qry_s"],
                  inputs["rm_qry_s"], inputs["rv_qry_s"], neg=True)
    wp, bp = fold(inputs["W_p"], inputs["g_p"], inputs["b_p"],
                  inputs["rm_p"], inputs["rv_p"], scale=1.0 / M)
    wpr, bpr = fold(inputs["W_prt"], inputs["g_prt"], inputs["b_prt"],
                    inputs["rm_prt"], inputs["rv_prt"], neg=True)
    biases = np.stack([bq, bs, bp, bpr]).astype(np.float32)  # [4, 640]

    def conv_matrix(cw):
        # rows: 0-24 = ch0 (max) taps, 32-56 = ch1 (mean) taps scaled 1/C,
        # rows 25-31 and 57-63 zero (partition-32 alignment for engines)
        cw = np.asarray(cw, np.float32)
        K = np.zeros((64, M), np.float32)
        for ch in range(2):
            base = 0 if ch == 0 else 32
            scale = 1.0 if ch == 0 else 1.0 / C
            for v in range(5):
                for u in range(5):
                    for y in range(5):
                        for x in range(5):
                            ky, kx = v - y + 1, u - x + 1
                            if 0 <= ky < 3 and 0 <= kx < 3:
                                K[base + v * 5 + u, y * 5 + x] = \
                                    cw[0, ch, ky, kx] * scale
        return np.ascontiguousarray(K, np.float32)

    kq = conv_matrix(inputs["cw_q"])
    kp = conv_matrix(inputs["cw_p"])

    spt = np.asarray(inputs["spt"], np.float32).reshape(WS, C, M)
    sptT = np.ascontiguousarray(spt.transpose(1, 0, 2).reshape(C, WS * M))

    cb = np.zeros((128, 2), np.float32)
    cb[:, 0] = np.float32(np.asarray(inputs["cb_q"]).reshape(-1)[0])
    cb[:, 1] = np.float32(np.asarray(inputs["cb_p"]).reshape(-1)[0])

    shared = dict(
        sptT=sptT, wq=wq, ws=ws, wp=wp, wpr=wpr, biases=biases,
        kq=kq, kp=kp,
        ones_row=np.ones((1, 128), np.float32),
        ones_col=np.ones((128, 1), np.float32),
        ident=np.eye(128, dtype=np.float32),
        cb=cb,
    )
    qry = np.ascontiguousarray(
        np.asarray(inputs["qry"], np.float32).reshape(B, C * M))
    in_maps = []
    for core in range(NCORES):
        m = dict(shared)
        m["q"] = np.ascontiguousarray(qry[core * BS:(core + 1) * BS])
        in_maps.append(m)
    return in_maps


def _run(inputs, trace=False, **kwargs):
    from concourse.bass_utils import run_bass_kernel_spmd

    if "prog" not in _PROG_CACHE:
        _PROG_CACHE["prog"] = _build_program()
    nc = _PROG_CACHE["prog"]

    in_maps = _prep_host(inputs)
    res = run_bass_kernel_spmd(nc, in_maps, list(range(NCORES)),
                               trace=trace, **kwargs)
    out = np.concatenate([r["out"] for r in res.results], axis=1)
    return np.ascontiguousarray(out, np.float32), res


def kernel(**inputs):
    out, _ = _run(inputs, trace=False)
    return out


def _timed_run(inputs, iters=20):
    """Mirror bass2jax.run_bass_via_pjrt's multi-core path, but keep inputs
    device-resident and recycle donated output buffers so repeated calls
    measure (dispatch + kernel exec) only. The kernel writes every output
    element, so recycled (non-zero) output buffers do not affect results."""
    import time
    import jax
    import numpy as np_
    from jax.sharding import Mesh, PartitionSpec
    from jax.experimental.shard_map import shard_map
    from concourse import mybir, bass2jax
    from concourse.bass2jax import _bass_exec_p, install_neuronx_cc_hook

    repeat = getattr(_timed_run, "repeat", 1)
    key = f"prog{repeat}"
    if key not in _PROG_CACHE:
        _PROG_CACHE[key] = _build_program(repeat)
    nc = _PROG_CACHE[key]
    install_neuronx_cc_hook()
    in_maps = _prep_host(inputs)
    n_cores = NCORES

    pid_name = nc.partition_id_tensor.name if nc.partition_id_tensor else None
    in_names, out_names, out_avals, zero_outs = [], [], [], []
    for alloc in nc.m.functions[0].allocations:
        if not isinstance(alloc, mybir.MemoryLocationSet):
            continue
        name = alloc.memorylocations[0].name
        if alloc.kind == "ExternalInput":
            if name != pid_name:
                in_names.append(name)
        elif alloc.kind == "ExternalOutput":
            out_names.append(name)
            shape = tuple(alloc.tensor_shape)
            dtype = mybir.dt.np(alloc.dtype)
            out_avals.append(jax.core.ShapedArray(shape, dtype))
            zero_outs.append(np_.zeros(shape, dtype))
    n_params = len(in_names)
    n_outs = len(out_avals)
    all_names = in_names + out_names
    if pid_name is not None:
        all_names = all_names + [pid_name]
    donate = tuple(range(n_params, n_params + n_outs))

    n_chain = getattr(_timed_run, "n_chain", 1)

    def _body(*args):
        ins = list(args[:n_params])
        outs = list(args[n_params:])
        for _ in range(n_chain):
            operands = ins + outs
            if pid_name is not None:
                operands.append(bass2jax.partition_id_tensor())
            outs = list(_bass_exec_p.bind(
                *operands,
                out_avals=tuple(out_avals),
                in_names=tuple(all_names),
                out_names=tuple(out_names),
                lowering_input_output_aliases=(),
                sim_require_finite=True,
                sim_require_nnan=True,
                nc=nc,
            ))
        return tuple(outs)

    devices = jax.devices()[:n_cores]
    mesh = Mesh(np_.asarray(devices), ("core",))
    sharded = jax.jit(
        shard_map(_body, mesh=mesh,
                  in_specs=(PartitionSpec("core"),) * (n_params + n_outs),
                  out_specs=(PartitionSpec("core"),) * n_outs,
                  check_rep=False),
        donate_argnums=donate, keep_unused=True,
    )
    per_core = [[np_.asarray(m[name]) for name in in_names] for m in in_maps]
    concat_in = [
        np_.concatenate([per_core[c][i] for c in range(n_cores)], axis=0)
        for i in range(n_params)
    ]
    concat_zeros = [
        np_.zeros((n_cores * z.shape[0], *z.shape[1:]), z.dtype)
        for z in zero_outs
    ]
    sh_in = jax.sharding.NamedSharding(mesh, PartitionSpec("core"))
    dev_in = [jax.device_put(a, sh_in) for a in concat_in]
    outs = sharded(*dev_in, *[jax.device_put(z, sh_in) for z in concat_zeros])
    jax.block_until_ready(outs)
    times = []
    for _ in range(iters):
        t0 = time.perf_counter()
        outs = sharded(*dev_in, *outs)
        jax.block_until_ready(outs)
        times.append(time.perf_counter() - t0)
    out = np_.asarray(outs[0]).reshape(n_cores, WS, BS, C)
    out = np_.concatenate([out[c] for c in range(n_cores)], axis=1)
    return np_.ascontiguousarray(out, np_.float32), times


# revision 22
# speedup vs baseline: 13.6999x; 13.6999x over previous
"""Trainium2 Bass kernel for nn_CSIW (prototype-scaling network).

Data-parallel over qry batch (4096 -> 8 cores x 512), spt + params
replicated. Per core: 4 batch-tiles of 128 rows (partition = batch,
free = (c, m) = 640*25). Engine split per tile:
  DVE    : mean-over-m reduce, qs = q*qg (bcast TT, in place), dist reduce-m
  ACT    : per-m fused Square(qs - q2) via per-partition bias AP, per-m
           sum-over-c via accum_out, tanh/sigmoid, PSUM->SBUF copies
  GPSIMD : max-over-c via strided tensor_tensor max tree
  PE     : all matmuls (BN-folded linears; conv as [50,25] matmul against a
           host-built kernel matrix; transposes; output 0.5wp+0.5wq via
           identity-copy + K=1 ones matmuls into PSUM, DMA'd straight out)
The tiny prt (support) branch runs once per core in channel-major layout.
"""

import numpy as np
from contextlib import ExitStack

C = 640
M = 25
WS = 25          # way * shot
B = 4096
NCORES = 8
BS = B // NCORES # 512 batch rows per core
NB = 128         # batch rows per tile
NT = BS // NB    # 4 tiles
CH = 320         # cout half for matmul free-dim chunks
CC = 5           # 640 / 128 channel chunks
EPS = 1e-5

_PROG_CACHE = {}


def _build_program(repeat=1):
    import concourse.bass as bass
    import concourse.bacc as bacc
    import concourse.tile as tile
    from concourse import mybir

    f32 = mybir.dt.float32
    f32r = mybir.dt.float32r
    Alu = mybir.AluOpType
    Act = mybir.ActivationFunctionType
    AX = mybir.AxisListType

    nc = bacc.Bacc("TRN2", target_bir_lowering=False, debug=False)

    # ---- DRAM I/O ----
    q_d = nc.dram_tensor("q", [BS, C * M], f32, kind="ExternalInput").ap()
    sptT_d = nc.dram_tensor("sptT", [C, WS * M], f32, kind="ExternalInput").ap()
    wq_d = nc.dram_tensor("wq", [C, C], f32, kind="ExternalInput").ap()
    ws_d = nc.dram_tensor("ws", [C, C], f32, kind="ExternalInput").ap()
    wp_d = nc.dram_tensor("wp", [C, C], f32, kind="ExternalInput").ap()
    wpr_d = nc.dram_tensor("wpr", [C, C], f32, kind="ExternalInput").ap()
    bias_d = nc.dram_tensor("biases", [4, C], f32, kind="ExternalInput").ap()
    kq_d = nc.dram_tensor("kq", [64, M], f32, kind="ExternalInput").ap()
    kp_d = nc.dram_tensor("kp", [64, M], f32, kind="ExternalInput").ap()
    ones_d = nc.dram_tensor("ones_row", [1, 128], f32, kind="ExternalInput").ap()
    onesc_d = nc.dram_tensor("ones_col", [128, 1], f32, kind="ExternalInput").ap()
    ident_d = nc.dram_tensor("ident", [128, 128], f32, kind="ExternalInput").ap()
    cb_d = nc.dram_tensor("cb", [128, 2], f32, kind="ExternalInput").ap()
    out_d = nc.dram_tensor("out", [WS, BS, C], f32, kind="ExternalOutput").ap()

    with tile.TileContext(nc) as tc, ExitStack() as ctx:
        cpool = ctx.enter_context(tc.tile_pool(name="consts", bufs=1))
        qpool = ctx.enter_context(tc.tile_pool(name="qdata", bufs=2))

        def cld(shape, name, src):
            t = cpool.tile(shape, f32, name=name, tag=name)
            nc.sync.dma_start(t[:], src)
            return t

        ones_sb = cld([1, 128], "ones_sb", ones_d[:, :])
        onesc_sb = cld([128, 1], "onesc_sb", onesc_d[:, :])
        ident_sb = cld([128, 128], "ident_sb", ident_d[:, :])
        kq_sb = cld([64, 25], "kq_sb", kq_d[:, :])
        kp_sb = cld([64, 25], "kp_sb", kp_d[:, :])
        cb_sb = cld([128, 2], "cb_sb", cb_d[:, :])
        ident_r = cpool.tile([128, 128], f32r, name="ident_r", tag="ident_r")
        nc.vector.tensor_copy(ident_r[:], ident_sb[:])
        ones_r = cpool.tile([1, 128], f32r, name="ones_r", tag="ones_r")
        nc.vector.tensor_copy(ones_r[:], ones_sb[:])
        bias_t = []
        for i in range(4):
            bt = cpool.tile([1, C], f32, name=f"bias{i}", tag=f"bias{i}")
            nc.sync.dma_start(bt[0:1, :], bias_d[i:i + 1, :])
            bias_t.append(bt)

        wq_sb = cpool.tile([128, CC * C], f32, name="wq_sb", tag="wq_sb")
        nc.sync.dma_start(
            wq_sb[:].rearrange("p (a n) -> p a n", a=CC),
            wq_d.rearrange("(a p) n -> p a n", p=128),
        )
        ws_sb = cpool.tile([128, CC * C], f32, name="ws_sb", tag="ws_sb")
        nc.sync.dma_start(
            ws_sb[:].rearrange("p (a n) -> p a n", a=CC),
            ws_d.rearrange("(a p) n -> p a n", p=128),
        )
        wph = cpool.tile([WS, C], f32r, name="wph", tag="wph")  # 0.5*w_prt

        # =========================================================
        # prt (support) branch, channel-major chunks [128c, (s,m)]
        # =========================================================
        with tc.tile_pool(name="prts", bufs=1) as ppool, \
             tc.tile_pool(name="pp_t", bufs=2, space="PSUM") as pp_t, \
             tc.tile_pool(name="pp_mm", bufs=2, space="PSUM") as pp_mm, \
             tc.tile_pool(name="pp_out", bufs=4, space="PSUM") as pp_out:

            wfull = ppool.tile([128, CC * C], f32, name="wfull", tag="wfull")
            nc.sync.dma_start(
                wfull[:].rearrange("p (a n) -> p a n", a=CC),
                wp_d.rearrange("(a p) n -> p a n", p=128),
            )
            wf3 = wfull[:].rearrange("p (a n) -> p a n", a=CC)
            P_sb = ppool.tile([128, CC * WS * M], f32, name="P_sb", tag="P_sb")
            nc.sync.dma_start(
                P_sb[:].rearrange("p (a f) -> p a f", a=CC),
                sptT_d.rearrange("(a p) f -> p a f", p=128),
            )
            P3 = P_sb[:].rearrange("p (a f) -> p a f", a=CC)        # [128,5,625]
            P4 = P_sb[:].rearrange("p (a s m) -> p a s m", a=CC, s=WS)

            # mean over m (1/25 folded into wp host-side)
            xqp = ppool.tile([128, CC * WS], f32, name="xqp", tag="xqp")
            for cc in range(CC):
                nc.vector.reduce_sum(
                    xqp[:, cc * WS:(cc + 1) * WS], P4[:, cc], axis=AX.X)

            # qg_p = 1 + tanh(Wp @ xq + b), channel-major [128c, 25s]
            qgp = ppool.tile([128, CC * WS], f32, name="qgp", tag="qgp")
            for co in range(CC):
                ps_g = pp_mm.tile([128, WS], f32, name="ps_g", tag="mm")
                for ci in range(CC):
                    nc.tensor.matmul(
                        ps_g[:], wf3[:, ci, co * 128:(co + 1) * 128],
                        xqp[:, ci * WS:(ci + 1) * WS],
                        start=(ci == 0), stop=False)
                nc.tensor.matmul(
                    ps_g[:], bias_t[2][0:1, co * 128:(co + 1) * 128],
                    ones_sb[0:1, 0:WS], start=False, stop=True)
                tgp = ppool.tile([128, WS], f32, name="tgp", tag="tgp")
                nc.scalar.activation(tgp[:], ps_g[:], Act.Tanh)
                nc.vector.tensor_scalar(
                    qgp[:, co * WS:(co + 1) * WS], tgp[:], 1.0, None, Alu.add)

            # qs_p = prt * qg (in place), bcast over m
            for cc in range(CC):
                nc.vector.tensor_tensor(
                    P4[:, cc], P4[:, cc],
                    qgp[:, cc * WS:(cc + 1) * WS].unsqueeze(2).broadcast_to(
                        [128, WS, M]),
                    op=Alu.mult)

            # max over c: cross-chunk folds + partition tree (ping-pong)
            ms = ppool.tile([128, WS * M], f32, name="ms", tag="ms")
            mt = ppool.tile([128, WS * M], f32, name="mt", tag="mt")
            nc.vector.tensor_tensor(ms[:], P3[:, 0], P3[:, 1], op=Alu.max)
            nc.vector.tensor_tensor(mt[:], ms[:], P3[:, 2], op=Alu.max)
            nc.vector.tensor_tensor(ms[:], mt[:], P3[:, 3], op=Alu.max)
            nc.vector.tensor_tensor(mt[:], ms[:], P3[:, 4], op=Alu.max)
            # partition reduce via gpsimd (walrus forbids partition-offset
            # TensorTensor operands)
            from concourse import bass_isa
            nc.gpsimd.partition_all_reduce(
                ms[:], mt[:], channels=128, reduce_op=bass_isa.ReduceOp.max)
            cur = ms
            # cur[0:1, :] = channel max, layout (s, m)

            # sum over c via PE ones-column matmuls -> [1, 625]
            ps_s1 = pp_mm.tile([1, 320], f32, name="ps_s1", tag="mm")
            for cc in range(CC):
                nc.tensor.matmul(ps_s1[:], onesc_sb[:, 0:1], P3[:, cc, 0:320],
                                 start=(cc == 0), stop=(cc == CC - 1))
            ps_s2 = pp_mm.tile([1, WS * M - 320], f32, name="ps_s2", tag="mm")
            for cc in range(CC):
                nc.tensor.matmul(ps_s2[:], onesc_sb[:, 0:1], P3[:, cc, 320:],
                                 start=(cc == 0), stop=(cc == CC - 1))
            sumrow = ppool.tile([1, WS * M], f32, name="sumrow", tag="sumrow")
            nc.scalar.copy(sumrow[0:1, 0:320], ps_s1[:])
            nc.scalar.copy(sumrow[0:1, 320:], ps_s2[:])

            # pooledT [50, 25]: rows 0-24 max over (q=m), rows 25-49 sum
            # route: flat [1,625] -> [25s,25q] DMA -> PE transpose -> [25q,25s]
            pooledT_p = ppool.tile([64, WS], f32, name="pooledT_p",
                                   tag="pooledT_p")
            tmp_mx = ppool.tile([WS, M], f32, name="tmp_mx", tag="tmp_mx")
            nc.sync.dma_start(tmp_mx[:, :], cur[0:1, :])
            tmp_sm = ppool.tile([WS, M], f32, name="tmp_sm", tag="tmp_sm")
            nc.sync.dma_start(tmp_sm[:, :], sumrow[0:1, :])
            nc.gpsimd.memset(pooledT_p[:, :], 0.0)
            ps_tm = pp_t.tile([M, WS], f32, name="ps_tm", tag="t")
            nc.tensor.transpose(ps_tm[:], tmp_mx[:, :], ident_sb[0:WS, 0:M])
            nc.scalar.copy(pooledT_p[0:WS, :], ps_tm[:])
            ps_ts = pp_t.tile([M, WS], f32, name="ps_ts", tag="t")
            nc.tensor.transpose(ps_ts[:], tmp_sm[:, :], ident_sb[0:WS, 0:M])
            nc.scalar.copy(pooledT_p[32:32 + WS, :], ps_ts[:])

            # conv as matmul -> [25s, 25p]; sigmoid(+cb)
            ps_c = pp_mm.tile([WS, M], f32, name="ps_c", tag="mm")
            nc.tensor.matmul(ps_c[:], pooledT_p[:, :], kp_sb[:, :],
                             start=True, stop=True)
            prt2sb = ppool.tile([WS, M], f32, name="prt2sb", tag="prt2sb")
            nc.scalar.activation(prt2sb[:], ps_c[:], Act.Sigmoid,
                                 bias=cb_sb[0:WS, 1:2])

            # flatten [25,25] -> [1,625]; broadcast to 128 partitions via PE
            prow = ppool.tile([1, WS * M], f32, name="prow", tag="prow")
            nc.sync.dma_start(prow[0:1, :], prt2sb[:, :])
            ps_b1 = pp_out.tile([128, 320], f32, name="ps_b1", tag="o")
            nc.tensor.matmul(ps_b1[:], ones_sb[0:1, :], prow[0:1, 0:320],
                             start=True, stop=True)
            ps_b2 = pp_out.tile([128, WS * M - 320], f32, name="ps_b2", tag="o")
            nc.tensor.matmul(ps_b2[:], ones_sb[0:1, :], prow[0:1, 320:],
                             start=True, stop=True)
            prt2b = ppool.tile([128, WS * M], f32, name="prt2b", tag="prt2b")
            nc.scalar.copy(prt2b[:, 0:320], ps_b1[:])
            nc.scalar.copy(prt2b[:, 320:], ps_b2[:])

            # dist1[c, s] = sum_m (qs_p - prt2b)^2
            dist1 = ppool.tile([128, CC * WS], f32, name="dist1", tag="dist1")
            for cc in range(CC):
                nc.vector.tensor_tensor(ms[:], P3[:, cc], prt2b[:],
                                        op=Alu.subtract)
                nc.scalar.activation(mt[:], ms[:], Act.Square)
                nc.vector.reduce_sum(
                    dist1[:, cc * WS:(cc + 1) * WS],
                    mt[:].rearrange("p (s m) -> p s m", s=WS), axis=AX.X)

            # w_prt half = 0.5 + 0.5*tanh(Wpr_neg @ dist1 + b)
            nc.sync.dma_start(
                wfull[:].rearrange("p (a n) -> p a n", a=CC),
                wpr_d.rearrange("(a p) n -> p a n", p=128),
            )
            wphT = ppool.tile([128, CC * WS], f32, name="wphT", tag="wphT")
            for co in range(CC):
                ps_w = pp_mm.tile([128, WS], f32, name="ps_w", tag="mm")
                for ci in range(CC):
                    nc.tensor.matmul(
                        ps_w[:], wf3[:, ci, co * 128:(co + 1) * 128],
                        dist1[:, ci * WS:(ci + 1) * WS],
                        start=(ci == 0), stop=False)
                nc.tensor.matmul(
                    ps_w[:], bias_t[3][0:1, co * 128:(co + 1) * 128],
                    ones_sb[0:1, 0:WS], start=False, stop=True)
                tpr = ppool.tile([128, WS], f32, name="tpr", tag="tpr")
                nc.scalar.activation(tpr[:], ps_w[:], Act.Tanh)
                nc.vector.tensor_scalar(
                    wphT[:, co * WS:(co + 1) * WS], tpr[:], 0.5, 0.5,
                    Alu.mult, Alu.add)
            # transpose [128c, 25s] chunks -> wph [25, 640]
            for cc in range(CC):
                ps_tp = pp_t.tile([WS, 128], f32, name="ps_tp", tag="t")
                nc.tensor.transpose(ps_tp[:], wphT[:, cc * WS:(cc + 1) * WS],
                                    ident_sb[:])
                nc.scalar.copy(wph[0:WS, cc * 128:(cc + 1) * 128], ps_tp[:])

        # =========================================================
        # batch tiles
        # =========================================================
        with tc.tile_pool(name="main", bufs=2) as mpool, \
             tc.tile_pool(name="bp_t", bufs=1, space="PSUM") as bp_t, \
             tc.tile_pool(name="bp_mm", bufs=3, space="PSUM") as bp_mm, \
             tc.tile_pool(name="bp_out", bufs=2, space="PSUM") as bp_out:

            wq3 = wq_sb[:].rearrange("p (a n) -> p a n", a=CC)
            ws3 = ws_sb[:].rearrange("p (a n) -> p a n", a=CC)

            def emit_out(ti, wqh):
                # out[s, b, c] = wqh[b, c] + wph[s, c]; PE -> PSUM -> SBUF -> DRAM
                for s in range(WS):
                    wrow = mpool.tile([1, C], f32r, name="wrow", tag="wrow",
                                      bufs=2)
                    nc.scalar.dma_start(wrow[0:1, :], wph[s:s + 1, :])
                    out_sb = mpool.tile([128, C], f32, name="out_sb",
                                        tag="out_sb", bufs=3)
                    for h in range(2):
                        ps_o = bp_out.tile([128, CH], f32, name="ps_o",
                                           tag="o", bufs=4)
                        nc.tensor.matmul(ps_o[:],
                                         ident_r[:],
                                         wqh[:, h * CH:(h + 1) * CH],
                                         start=True, stop=False)
                        nc.tensor.matmul(ps_o[:],
                                         ones_r[0:1, :],
                                         wrow[0:1, h * CH:(h + 1) * CH],
                                         start=False, stop=True)
                        if s % 2 == 0:
                            nc.vector.tensor_copy(
                                out_sb[:, h * CH:(h + 1) * CH], ps_o[:])
                        else:
                            nc.scalar.copy(
                                out_sb[:, h * CH:(h + 1) * CH], ps_o[:])
                    eng = nc.sync if s % 2 == 0 else nc.scalar
                    eng.dma_start(
                        out_d[s, ti * NB:(ti + 1) * NB, :], out_sb[:])

            pending = []

            def compute_main(ti):
                Q = qpool.tile([128, C * M], f32, name="Q", tag="Q")
                for ck in range(4):
                    nc.sync.dma_start(
                        Q[:, ck * 4000:(ck + 1) * 4000],
                        q_d[ti * NB:(ti + 1) * NB, ck * 4000:(ck + 1) * 4000])
                Q3 = Q[:].rearrange("p (c m) -> p c m", m=M)   # [128,640,25]

                # xq = sum_m q  (1/25 folded into wq host-side)
                xq = mpool.tile([128, C], f32, name="xq", tag="xq")
                nc.vector.reduce_sum(xq[:], Q3, axis=AX.X)
                # transpose xq -> xqT chunks [128c, 128b]
                xqT = mpool.tile([128, C], f32, name="xqT", tag="xqT", bufs=1)
                for cc in range(CC):
                    ps_t = bp_t.tile([128, 128], f32, name="ps_t", tag="t")
                    nc.tensor.transpose(
                        ps_t[:], xq[:, cc * 128:(cc + 1) * 128], ident_sb[:])
                    nc.scalar.copy(xqT[:, cc * 128:(cc + 1) * 128], ps_t[:])
                # qg = 1 + tanh(xq @ wq + b)
                qg = mpool.tile([128, C], f32, name="qg", tag="qg")
                for h in range(2):
                    ps_g = bp_mm.tile([128, CH], f32, name="ps_g", tag="mm")
                    for cc in range(CC):
                        nc.tensor.matmul(
                            ps_g[:], xqT[:, cc * 128:(cc + 1) * 128],
                            wq3[:, cc, h * CH:(h + 1) * CH],
                            start=(cc == 0), stop=False)
                    nc.tensor.matmul(
                        ps_g[:], ones_sb[0:1, 0:128],
                        bias_t[0][0:1, h * CH:(h + 1) * CH],
                        start=False, stop=True)
                    nc.scalar.activation(qg[:, h * CH:(h + 1) * CH], ps_g[:],
                                         Act.Tanh)
                nc.vector.tensor_scalar(qg[:], qg[:], 1.0, None, Alu.add)

                # qs = q * qg (in place, bcast over m) on gpsimd
                nc.gpsimd.tensor_tensor(
                    Q3, Q3, qg[:].unsqueeze(2).broadcast_to([128, C, M]),
                    op=Alu.mult)

                # pooled [128, 50]: cols 0-24 max-over-c, cols 25-49 sum
                pooled = mpool.tile([128, 64], f32, name="pooled", tag="pooled")
                nc.gpsimd.memset(pooled[:, :], 0.0)
                # max over c: strided reduce, view (m outer, c inner)
                Qmc = Q[:].rearrange("p (c m) -> p m c", m=M)
                nc.vector.reduce_max(pooled[:, 0:M], Qmc, axis=AX.X)
                # sum over c: 25 ACT copies with accum_out
                ascr = mpool.tile([128, C], f32, name="ascr", tag="ascr",
                                  bufs=1)
                for m in range(M):
                    nc.scalar.activation(
                        ascr[:], Q3[:, :, m], Act.Copy,
                        accum_out=pooled[:, 32 + m:32 + m + 1])

                # pooledT = pooled.T -> conv matmul -> q2neg
                ps_pt = bp_t.tile([64, 128], f32, name="ps_pt", tag="t")
                nc.tensor.transpose(ps_pt[:], pooled[:], ident_sb[:])
                pooledT = mpool.tile([64, 128], f32, name="pooledT",
                                     tag="pooledT")
                nc.scalar.copy(pooledT[:], ps_pt[:])
                ps_cv = bp_mm.tile([128, M], f32, name="ps_cv", tag="mm")
                nc.tensor.matmul(ps_cv[:], pooledT[:], kq_sb[:, :],
                                 start=True, stop=True)
                q2t = mpool.tile([128, M], f32, name="q2t", tag="q2t")
                nc.scalar.activation(q2t[:], ps_cv[:], Act.Sigmoid,
                                     bias=cb_sb[:, 0:1])

                # dist: (qs - q2)^2 in place on Q (gpsimd sub, ACT square)
                nc.gpsimd.tensor_tensor(
                    Q3, Q3,
                    q2t[:].unsqueeze(1).broadcast_to([128, C, M]),
                    op=Alu.subtract)
                nc.scalar.activation(Q[:], Q[:], Act.Square)
                dist2 = mpool.tile([128, C], f32, name="dist2", tag="dist2")
                nc.vector.reduce_sum(dist2[:], Q3, axis=AX.X)

                # transpose dist2; wqh = 0.5 + 0.5*tanh(ws_neg @ dist2 + b)
                dist2T = mpool.tile([128, C], f32, name="dist2T", tag="dist2T",
                                    bufs=1)
                for cc in range(CC):
                    ps_t2 = bp_t.tile([128, 128], f32, name="ps_t2", tag="t")
                    nc.tensor.transpose(
                        ps_t2[:], dist2[:, cc * 128:(cc + 1) * 128],
                        ident_sb[:])
                    nc.scalar.copy(dist2T[:, cc * 128:(cc + 1) * 128],
                                   ps_t2[:])
                wqh = mpool.tile([128, C], f32r, name="wqh", tag="wqh")
                for h in range(2):
                    ps_w = bp_mm.tile([128, CH], f32, name="ps_w", tag="mm")
                    for cc in range(CC):
                        nc.tensor.matmul(
                            ps_w[:], dist2T[:, cc * 128:(cc + 1) * 128],
                            ws3[:, cc, h * CH:(h + 1) * CH],
                            start=(cc == 0), stop=False)
                    nc.tensor.matmul(
                        ps_w[:], ones_sb[0:1, 0:128],
                        bias_t[1][0:1, h * CH:(h + 1) * CH],
                        start=False, stop=True)
                    nc.scalar.activation(wqh[:, h * CH:(h + 1) * CH], ps_w[:],
                                         Act.Tanh)
                nc.vector.tensor_scalar(wqh[:], wqh[:], 0.5, 0.5,
                                        Alu.mult, Alu.add)
                return wqh

            for ti in [t for _ in range(repeat) for t in range(NT)]:
                wqh_t = compute_main(ti)
                pending.append((ti, wqh_t))
                if len(pending) > 1:
                    emit_out(*pending.pop(0))
            while pending:
                emit_out(*pending.pop(0))
    nc.compile()
    return nc


def _prep_host(inputs):
    """Fold BN into weights, build conv kernel matrices, layout transforms."""
    def fold(W, g, b, rm, rv, scale=1.0, neg=False):
        W = np.asarray(W, np.float32)
        g = np.asarray(g, np.float32)
        b = np.asarray(b, np.float32)
        rm = np.asarray(rm, np.float32)
        rv = np.asarray(rv, np.float32)
        alpha = g / np.sqrt(rv + EPS)
        Wt = (W.T * alpha[None, :]) * scale       # [cin, cout]
        if neg:
            Wt = -Wt
        bt = b - rm * alpha
        return np.ascontiguousarray(Wt, np.float32), bt.astype(np.float32)

    wq, bq = fold(inputs["W_q"], inputs["g_q"], inputs["b_q"],
                  inputs["rm_q"], inputs["rv_q"], scale=1.0 / M)
    ws, bs = fold(inputs["W_qry_s"], inputs["g_qry_s"], inputs["b_qry_s"],
                  inputs["rm_qry_s"], inputs["rv_qry_s"], neg=True)
    wp, bp = fold(inputs["W_p"], inputs["g_p"], inputs["b_p"],
                  inputs["rm_p"], inputs["rv_p"], scale=1.0 / M)
    wpr, bpr = fold(inputs["W_prt"], inputs["g_prt"], inputs["b_prt"],
                    inputs["rm_prt"], inputs["rv_prt"], neg=True)
    biases = np.stack([bq, bs, bp, bpr]).astype(np.float32)  # [4, 640]

    def conv_matrix(cw):
        # rows: 0-24 = ch0 (max) taps, 32-56 = ch1 (mean) taps scaled 1/C,
        # rows 25-31 and 57-63 zero (partition-32 alignment for engines)
        cw = np.asarray(cw, np.float32)
        K = np.zeros((64, M), np.float32)
        for ch in range(2):
            base = 0 if ch == 0 else 32
            scale = 1.0 if ch == 0 else 1.0 / C
            for v in range(5):
                for u in range(5):
                    for y in range(5):
                        for x in range(5):
                            ky, kx = v - y + 1, u - x + 1
                            if 0 <= ky < 3 and 0 <= kx < 3:
                                K[base + v * 5 + u, y * 5 + x] = \
                                    cw[0, ch, ky, kx] * scale
        return np.ascontiguousarray(K, np.float32)

    kq = conv_matrix(inputs["cw_q"])
    kp = conv_matrix(inputs["cw_p"])

    spt = np.asarray(inputs["spt"], np.float32).reshape(WS, C, M)
    sptT = np.ascontiguousarray(spt.transpose(1, 0, 2).reshape(C, WS * M))

    cb = np.zeros((128, 2), np.float32)
    cb[:, 0] = np.float32(np.asarray(inputs["cb_q"]).reshape(-1)[0])
    cb[:, 1] = np.float32(np.asarray(inputs["cb_p"]).reshape(-1)[0])

    shared = dict(
        sptT=sptT, wq=wq, ws=ws, wp=wp, wpr=wpr, biases=biases,
        kq=kq, kp=kp,
        ones_row=np.ones((1, 128), np.float32),
        ones_col=np.ones((128, 1), np.float32),
        ident=np.eye(128, dtype=np.float32),
        cb=cb,
    )
    qry = np.ascontiguousarray(
        np.asarray(inputs["qry"], np.float32).reshape(B, C * M))
    in_maps = []
    for core in range(NCORES):
        m = dict(shared)
        m["q"] = np.ascontiguousarray(qry[core * BS:(core + 1) * BS])
        in_maps.append(m)
    return in_maps


def _run(inputs, trace=False, **kwargs):
    from concourse.bass_utils import run_bass_kernel_spmd

    if "prog" not in _PROG_CACHE:
        _PROG_CACHE["prog"] = _build_program()
    nc = _PROG_CACHE["prog"]

    in_maps = _prep_host(inputs)
    res = run_bass_kernel_spmd(nc, in_maps, list(range(NCORES)),
                               trace=trace, **kwargs)
    out = np.concatenate([r["out"] for r in res.results], axis=1)
    return np.ascontiguousarray(out, np.float32), res


def kernel(**inputs):
    out, _ = _run(inputs, trace=False)
    return out


def _timed_run(inputs, iters=20):
    """Mirror bass2jax.run_bass_via_pjrt's multi-core path, but keep inputs
    device-resident and recycle donated output buffers so repeated calls
    measure (dispatch + kernel exec) only. The kernel writes every output
    element, so recycled (non-zero) output buffers do not affect results."""
    import time
    import jax
    import numpy as np_
    from jax.sharding import Mesh, PartitionSpec
    from jax.experimental.shard_map import shard_map
    from concourse import mybir, bass2jax
    from concourse.bass2jax import _bass_exec_p, install_neuronx_cc_hook

    repeat = getattr(_timed_run, "repeat", 1)
    key = f"prog{repeat}"
    if key not in _PROG_CACHE:
        _PROG_CACHE[key] = _build_program(repeat)
    nc = _PROG_CACHE[key]
    install_neuronx_cc_hook()
    in_maps = _prep_host(inputs)
    n_cores = NCORES

    pid_name = nc.partition_id_tensor.name if nc.partition_id_tensor else None
    in_names, out_names, out_avals, zero_outs = [], [], [], []
    for alloc in nc.m.functions[0].allocations:
        if not isinstance(alloc, mybir.MemoryLocationSet):
            continue
        name = alloc.memorylocations[0].name
        if alloc.kind == "ExternalInput":
            if name != pid_name:
                in_names.append(name)
        elif alloc.kind == "ExternalOutput":
            out_names.append(name)
            shape = tuple(alloc.tensor_shape)
            dtype = mybir.dt.np(alloc.dtype)
            out_avals.append(jax.core.ShapedArray(shape, dtype))
            zero_outs.append(np_.zeros(shape, dtype))
    n_params = len(in_names)
    n_outs = len(out_avals)
    all_names = in_names + out_names
    if pid_name is not None:
        all_names = all_names + [pid_name]
    donate = tuple(range(n_params, n_params + n_outs))

    n_chain = getattr(_timed_run, "n_chain", 1)

    def _body(*args):
        ins = list(args[:n_params])
        outs = list(args[n_params:])
        for _ in range(n_chain):
            operands = ins + outs
            if pid_name is not None:
                operands.append(bass2jax.partition_id_tensor())
            outs = list(_bass_exec_p.bind(
                *operands,
                out_avals=tuple(out_avals),
                in_names=tuple(all_names),
                out_names=tuple(out_names),
                lowering_input_output_aliases=(),
                sim_require_finite=True,
                sim_require_nnan=True,
                nc=nc,
            ))
        return tuple(outs)

    devices = jax.devices()[:n_cores]
    mesh = Mesh(np_.asarray(devices), ("core",))
    sharded = jax.jit(
        shard_map(_body, mesh=mesh,
                  in_specs=(PartitionSpec("core"),) * (n_params + n_outs),
                  out_specs=(PartitionSpec("core"),) * n_outs,
                  check_rep=False),
        donate_argnums=donate, keep_unused=True,
    )
    per_core = [[np_.asarray(m[name]) for name in in_names] for m in in_maps]
    concat_in = [
        np_.concatenate([per_core[c][i] for c in range(n_cores)], axis=0)
        for i in range(n_params)
    ]
    concat_zeros = [
        np_.zeros((n_cores * z.shape[0], *z.shape[1:]), z.dtype)
        for z in zero_outs
    ]
    sh_in = jax.sharding.NamedSharding(mesh, PartitionSpec("core"))
    dev_in = [jax.device_put(a, sh_in) for a in concat_in]
    outs = sharded(*dev_in, *[jax.device_put(z, sh_in) for z in concat_zeros])
    jax.block_until_ready(outs)
    times = []
    for _ in range(iters):
        t0 = time.perf_counter()
        outs = sharded(*dev_in, *outs)
        jax.block_until_ready(outs)
        times.append(time.perf_counter() - t0)
    out = np_.asarray(outs[0]).reshape(n_cores, WS, BS, C)
    out = np_.concatenate([out[c] for c in range(n_cores)], axis=1)
    return np_.ascontiguousarray(out, np_.float32), times
